# revision 4
# baseline (speedup 1.0000x reference)
"""AffinityLoss on 8 TRN2 NeuronCores (Bass/Tile) — PE-reduced channels.

Math: 3x3-unfold affinity loss = mean BCE-with-logits between per-pixel 9x9
channel Gram matrices and label-equality maps, reformulated over 13 canonical
relative shifts s=(dr,dc) (see _combine for the host-side weighting):

    loss_sum = sum_s mult_s * sum_{y,x} wy_s(y) wx_s(x) * ln(1 + exp(w_s(y,x)))
    w_s = (1 - 2*E_s) * Corr_s,   Corr_s(y,x) = sum_c A[c,y,x] A[c,y+dr,x+dc]

Device pipeline per shift: DVE computes the 19-channel product tile
pb[128, 19, DW] in bf16 at the 2x rate (odd dc offsets included — no
odd-parity tile copies, halving input DMA vs v1); the channel reduction runs
on the otherwise-idle TensorEngine as 19 identity matmuls accumulating into a
PSUM bank (exact f32 accumulate); ScalarE applies Exp (scale=-1 for s=(0,0))
and Ln(bias=1, accum_out) straight from PSUM; label-sign for s!=0 is
is_equal (DVE) -> 1-2e (ScalarE affine) -> mult with PSUM corr (DVE).
Border wx deviations live within 2 cols of the edges; those columns are
exported exactly and corrected on host in f64 with the wy weighting and the
cross-core reduction.

Sharding: data-parallel, core k owns image k//2, rows [192*(k%2), +192).
Pass0 = 128 rows full width; pass1 = 64 rows x 2 x-half groups stacked on
partitions. Passes use separate tiles so pass1's DMA overlaps pass0's tail.
DGE launches are ~0.6us each and serialize per engine; T0's 19 single-channel
chunk launches are split across both HWDGE engines (sync + scalar) so the
full tile lands ~11us in, with T1/T2 queued behind on sync.
"""

import os
import sys

import numpy as np

for _p in ("/root/.axon_site", "/root/.axon_site/_ro/trn_rl_repo",
           "/root/.axon_site/_ro/pypackages"):
    if os.path.isdir(_p) and _p not in sys.path:
        sys.path.append(_p)

import ml_dtypes  # noqa: E402

N, C, H, W = 4, 19, 384, 384
K = 3
HP = WP = H - K + 1  # 382
N_CORES = 8
ROWS_PER_CORE = 192
SHIFTS = [(0, 0), (0, 1), (0, 2)] + [(dr, dc) for dr in (1, 2) for dc in (-2, -1, 0, 1, 2)]
NS = len(SHIFTS)  # 13
PASS_GEOM = [
    dict(row0=0, DW=W, TW=W + 8, groups=1),      # tiles [128, C, 392], data cols 2..385
    dict(row0=128, DW=194, TW=200, groups=2),    # 64 rows x 2 x-groups
]
GROUP_X0 = [0, 190]   # pass1 group g covers x in [190g, 190g+194)
COLS = 2 * NS        # one ln-accum column per (pass, shift)
BCOLS = 2 * NS * 4   # 4 exported border cols per (pass, shift)
LGROWS = 196


def _wx_profile(dc, x):
    w = np.zeros_like(x, dtype=np.float64)
    for ca in range(K):
        if 0 <= ca + dc < K:
            w += ((x - ca >= 0) & (x - ca < WP))
    return w


def _wy_profile(dr, y):
    w = np.zeros_like(y, dtype=np.float64)
    for ra in range(K):
        if 0 <= ra + dr < K:
            w += ((y - ra >= 0) & (y - ra < HP))
    return w


def _border_weights():
    """bw[p, (pass*NS+s)*4 + bi]: (wx_eff - wxc) at window cols {0,1,DW-2,DW-1}."""
    bw = np.zeros((128, 2 * NS * 4), np.float64)
    for pi, geom in enumerate(PASS_GEOM):
        DW = geom["DW"]
        for si, (dr, dc) in enumerate(SHIFTS):
            wxc = sum(1 for ca in range(K) if 0 <= ca + dc < K)
            for p in range(128):
                if pi == 0:
                    gx0, own_lo, own_hi = 0, 0, W
                else:
                    g = p // 64
                    gx0 = GROUP_X0[g]
                    own_lo, own_hi = (0, 192) if g == 0 else (192, W)
                for bi, j in enumerate((0, 1, DW - 2, DW - 1)):
                    x = gx0 + j
                    if own_lo <= x < own_hi and 0 <= x + dc < W and x < W:
                        wx = _wx_profile(dc, np.array([x]))[0]
                    else:
                        wx = 0.0
                    bw[p, (pi * NS + si) * 4 + bi] = wx - wxc
    return bw


_BW = None


def _host_inputs(logits, labels):
    in_maps = []
    for k in range(N_CORES):
        img, half = k // 2, k % 2
        g0 = half * ROWS_PER_CORE
        hi = min(H, g0 + LGROWS)
        lg = np.zeros((C, LGROWS, W), np.float32)
        lg[:, : hi - g0] = logits[img, :, g0:hi]
        lb = np.full((LGROWS, W), -1.0, np.float32)
        lb[: hi - g0] = labels[img, g0:hi].astype(np.float32)
        in_maps.append({
            "lg": lg.astype(ml_dtypes.bfloat16),
            "lb": lb.astype(ml_dtypes.bfloat16),
        })
    return in_maps


def _combine(accs_list, bcols_list):
    global _BW
    if _BW is None:
        _BW = _border_weights()
    total = 0.0
    for k in range(N_CORES):
        acc = accs_list[k].astype(np.float64)
        bc = bcols_list[k].astype(np.float64)
        g0 = (k % 2) * ROWS_PER_CORE
        for pi in range(2):
            p = np.arange(128)
            gy = g0 + p if pi == 0 else g0 + 128 + (p % 64)
            for si, (dr, dc) in enumerate(SHIFTS):
                mult = 1.0 if (dr, dc) == (0, 0) else 2.0
                wxc = float(sum(1 for ca in range(K) if 0 <= ca + dc < K))
                wy = _wy_profile(dr, gy)
                idx = pi * NS + si
                wb = _BW[:, idx * 4: idx * 4 + 4]
                full = acc[:, idx]
                border = (bc[:, idx * 4: idx * 4 + 4] * wb).sum(1)
                total += mult * np.sum(wy * (wxc * full + border))
    return total / (N * 81 * HP * WP)


_NC = None


def _build():
    global _NC
    if _NC is not None:
        return _NC
    from concourse import bacc, mybir
    import concourse.tile as tile
    from concourse.masks import make_identity

    f32 = mybir.dt.float32
    bf16 = mybir.dt.bfloat16
    Alu = mybir.AluOpType
    AF = mybir.ActivationFunctionType

    # All activations used here (Exp, Ln, Square, Copy) live together in the
    # "natural_log_exp_and_others" table set; filter the other sets'
    # membership so the table-load pass resolves them all to the one shared
    # set -> a single ACT_TABLE_LOAD.
    from concourse.hw_specs import get_activation_tables as _gat
    _keep = "natural_log_exp_and_others"
    _mine = {AF.Exp, AF.Ln, AF.Square, AF.Copy}

    def _gat_filtered(arch):
        t = _gat(arch)
        for name in t:
            if name != _keep:
                t[name] = t[name] - _mine
        return t

    bacc.get_activation_tables = _gat_filtered

    nc = bacc.Bacc("TRN2", target_bir_lowering=False, debug=False, num_devices=N_CORES)
    lg = nc.dram_tensor("lg", (C, LGROWS, W), bf16, kind="ExternalInput")
    lb = nc.dram_tensor("lb", (LGROWS, W), bf16, kind="ExternalInput")
    accs = nc.dram_tensor("accs", (128, COLS), f32, kind="ExternalOutput")
    bcols = nc.dram_tensor("bcols", (128, BCOLS), f32, kind="ExternalOutput")

    with tile.TileContext(nc) as tc:
        with tc.tile_pool(name="persist", bufs=1) as pool, \
             tc.tile_pool(name="work", bufs=4) as wpool, \
             tc.tile_pool(name="psum", bufs=8, space="PSUM") as ppool:
            accs_t = pool.tile([128, COLS], f32, name="accs_t")
            bcols_t = pool.tile([128, BCOLS], f32, name="bcols_t")
            eye = pool.tile([128, 128], bf16, name="eye")
            make_identity(nc, eye)

            for pi, geom in enumerate(PASS_GEOM):
                row0, DW, TW, groups = geom["row0"], geom["DW"], geom["TW"], geom["groups"]
                T, L = {}, {}
                for dr in range(K):
                    t = pool.tile([128, C, TW], bf16, tag=f"T{dr}_{pi}", name=f"T{dr}_{pi}")
                    lt = pool.tile([128, TW], bf16, tag=f"L{dr}_{pi}", name=f"L{dr}_{pi}")
                    nc.gpsimd.memset(t[:, :, 0:2], 0)
                    nc.gpsimd.memset(t[:, :, 2 + DW:TW], 0)
                    nc.gpsimd.memset(lt[:, 0:2], 0)
                    nc.gpsimd.memset(lt[:, 2 + DW:TW], 0)
                    T[dr], L[dr] = t, lt
                for dr in range(K):
                    eng = nc.sync
                    if groups == 1:
                        if dr == 0:
                            # split T0's 19 single-channel launches across
                            # both HWDGE engines so the full tile lands ~11us
                            for c in range(C):
                                e2 = nc.sync if c < 10 else nc.scalar
                                src = lg[c:c + 1, row0:row0 + 128, :].rearrange(
                                    "c y x -> y c x")
                                e2.dma_start(T[0][:, c:c + 1, 2:2 + DW], src)
                            nc.scalar.dma_start(L[0][:, 2:2 + DW],
                                                lb[row0:row0 + 128, :])
                            continue
                        bnds = [0, 3, 5, 8, 10, 12, 14, 17, C]
                        for c0, c1 in zip(bnds[:-1], bnds[1:]):
                            src = lg[c0:c1, row0 + dr:row0 + dr + 128, :].rearrange(
                                "c y x -> y c x")
                            eng.dma_start(T[dr][:, c0:c1, 2:2 + DW], src)
                        eng.dma_start(L[dr][:, 2:2 + DW],
                                      lb[row0 + dr:row0 + dr + 128, :])
                    else:
                        for g in range(groups):
                            x0 = GROUP_X0[g]
                            for c0, c1 in ((0, 5), (5, 10), (10, 14), (14, C)):
                                src = lg[c0:c1, row0 + dr:row0 + dr + 64, x0:x0 + DW
                                         ].rearrange("c y x -> y c x")
                                eng.dma_start(
                                    T[dr][64 * g:64 * g + 64, c0:c1, 2:2 + DW], src)
                            eng.dma_start(
                                L[dr][64 * g:64 * g + 64, 2:2 + DW],
                                lb[row0 + dr:row0 + dr + 64, x0:x0 + DW])

                egrp, shgrp = {}, {}
                for si in range(NS):
                    dr, dc = SHIFTS[si]
                    idx = pi * NS + si
                    o1 = 2 + dc
                    # first shift of each dr: compute the whole dr-group's
                    # label-equality + sign tiles in one instruction each
                    if si != 0 and dr not in egrp:
                        ns_g = 2 if dr == 0 else 5
                        off0 = 3 if dr == 0 else 0
                        e_g = wpool.tile([128, ns_g, DW], bf16, tag="e",
                                         name=f"e_{pi}_{dr}")
                        sh_g = wpool.tile([128, ns_g, DW], bf16, tag="sh",
                                          name=f"sh_{pi}_{dr}")
                        in0 = L[0][:, 2:2 + DW].unsqueeze(1).broadcast_to(
                            [128, ns_g, DW])
                        from concourse.ap import AP as _AP
                        base = L[dr][:, 0:DW]
                        in1 = _AP(tensor=base.tensor, offset=off0,
                                  ap=[[TW, 128], [1, ns_g], [1, DW]])
                        nc.vector.tensor_tensor(e_g, in0, in1, Alu.is_equal)
                        nc.scalar.activation(sh_g, e_g, AF.Identity,
                                             bias=1.0, scale=-2.0)
                        egrp[dr], shgrp[dr] = e_g, sh_g

                    pb = wpool.tile([128, C, DW], bf16, tag="pb", name=f"pb_{pi}_{si}")
                    u = wpool.tile([128, DW], f32, tag="u", name=f"u_{pi}_{si}")
                    l1 = wpool.tile([128, DW], f32, tag="l1", name=f"l1_{pi}_{si}")
                    corr = ppool.tile([128, DW], f32, tag="corr", name=f"corr_{pi}_{si}")

                    # products: chunk the dr==0 shifts over c so compute
                    # starts while the T0 chunks are still arriving
                    csplits = ((0, 5), (5, 10), (10, 14), (14, C)) \
                        if (dr == 0 and pi == 0) else ((0, C),)
                    for c0, c1 in csplits:
                        if si == 0:
                            # Corr_00 = sum_c A_c^2: products on ScalarE
                            nc.scalar.activation(
                                pb[:, c0:c1, :], T[0][:, c0:c1, 2:2 + DW], AF.Square)
                        else:
                            nc.vector.tensor_tensor(
                                pb[:, c0:c1, :], T[0][:, c0:c1, 2:2 + DW],
                                T[dr][:, c0:c1, o1:o1 + DW], Alu.mult)
                    # channel reduction on the TensorEngine: 19 identity
                    # matmuls accumulating into one PSUM bank (exact f32)
                    for c in range(C):
                        nc.tensor.matmul(corr, eye, pb[:, c, :],
                                         start=(c == 0), stop=(c == C - 1))

                    if si == 0:
                        # E == 1 everywhere: w = -corr, fold into Exp's scale
                        nc.scalar.activation(u[:, 0:DW], corr[:, 0:DW], AF.Exp,
                                             scale=-1.0)
                    else:
                        wt = wpool.tile([128, DW], bf16, tag="wt", name=f"wt_{pi}_{si}")
                        j = (si - 1) if dr == 0 else (dc + 2)
                        nc.vector.tensor_tensor(
                            wt[:, 0:DW], shgrp[dr][:, j, :], corr[:, 0:DW],
                            Alu.mult)
                        nc.scalar.activation(u[:, 0:DW], wt[:, 0:DW], AF.Exp)
                    nc.scalar.activation(
                        l1[:, 0:DW], u[:, 0:DW], AF.Ln, bias=1.0,
                        accum_out=accs_t[:, idx:idx + 1])
                    nc.scalar.copy(bcols_t[:, idx * 4:idx * 4 + 2], l1[:, 0:2])
                    nc.scalar.copy(bcols_t[:, idx * 4 + 2:idx * 4 + 4],
                                   l1[:, DW - 2:DW])

                # flush this pass's halves so the kernel tail is short
                nc.sync.dma_start(accs[:, pi * NS:(pi + 1) * NS],
                                  accs_t[:, pi * NS:(pi + 1) * NS])
                nc.sync.dma_start(bcols[:, pi * NS * 4:(pi + 1) * NS * 4],
                                  bcols_t[:, pi * NS * 4:(pi + 1) * NS * 4])

    nc.finalize()
    _NC = nc
    return nc


def kernel(logits, labels):
    nc = _build()
    in_maps = _host_inputs(np.asarray(logits, np.float32), np.asarray(labels))
    from concourse.bass_utils import run_bass_kernel_spmd
    res = run_bass_kernel_spmd(nc, in_maps, core_ids=list(range(N_CORES)))
    accs_list = [res.results[k]["accs"] for k in range(N_CORES)]
    bcols_list = [res.results[k]["bcols"] for k in range(N_CORES)]
    return np.array(_combine(accs_list, bcols_list), np.float32)
